# revision 7
# baseline (speedup 1.0000x reference)
"""Bahdanau-style attention kernel for Trainium2 (Bass/Tile), 8-core data parallel.

Reference computation (per batch b):
    proj_f = features @ W1 + b1            # [B, L, U]
    proj_h = hidden @ W2 + b2              # [B, U]
    score  = tanh(proj_f + proj_h[:,None]) # [B, L, U]
    logits = score @ V + bV                # [B, L, 1]
    attw   = softmax(logits, axis=1)       # [B, L, 1]
    ctx    = sum(attw * features, axis=1)  # [B, D]

Shapes: B=2048, L=64, D=256, U=512.  Sharding: batch split across 8 cores
(Bc=256 per core, rows = Bc*L = 16384).

Per-core layout strategy (see comments inline):
  - features loaded naturally [rows, D] (rows on partitions), then PE-transposed
    to featT [D, rows] for the W1 matmul (PE contracts the partition dim).
  - projT [U on partitions, rows free]: W1 chunks stationary, featT streams.
  - hidden projection proj_h is spread across each batch's 64 rows with a
    rank-8 indicator matmul directly into the projT PSUM accumulation.
  - tanh on ScalarE (PSUM -> SBUF), bias carries b1+b2.
  - logits: V-chunk stationary (M=1), scoreT streams, accumulate over 4 U chunks.
  - softmax without max-subtraction (logits are bounded by ||V||_1): exp with
    fused accum_out on ScalarE, reciprocal+scale on VectorE.
  - context: features-natural stationary, block-diagonal weight columns stream;
    all 512 output columns accumulate in a single PSUM bank, one drain.
"""

import sys

if "/opt/trn_rl_repo" not in sys.path:
    sys.path.insert(0, "/opt/trn_rl_repo")

import numpy as np

import concourse.bass as bass
import concourse.tile as tile
from concourse import bacc, mybir

F32 = mybir.dt.float32
F32R = mybir.dt.float32r

# Matmul dtype (float32r streams one row/cycle for N>=256; plain float32 is 4x
# slower).  Accuracy of f32r vs f32 is checked empirically by test.py.
MM = F32R

N_CORES = 8
B_FULL = 2048
L = 64
D = 256
U = 512

BC = B_FULL // N_CORES          # batches per core          = 256
ROWS = BC * L                   # rows per core             = 16384
RB = 2048                       # rows per block
NBLK = ROWS // RB               # blocks per core           = 8
CPB = RB // 128                 # 128-row chunks per block  = 16
SUB = 512                       # rows per matmul sub-chunk
NSUB = RB // SUB                # sub-chunks per block      = 4
DK = D // 128                   # D chunks                  = 2
UK = U // 128                   # U chunks                  = 4


def r(ap):
    """View an fp32 AP as float32r for full-rate PE streaming."""
    return ap.bitcast(MM)


def build_kernel():
    nc = bacc.Bacc("TRN2", target_bir_lowering=False, debug=False)

    feats_d = nc.dram_tensor("feats", [ROWS, D], F32R, kind="ExternalInput")
    hid_d = nc.dram_tensor("hid", [BC, U], F32R, kind="ExternalInput")
    w1_d = nc.dram_tensor("w1", [D, U], F32R, kind="ExternalInput")
    b1_d = nc.dram_tensor("b1", [U], F32, kind="ExternalInput")
    w2_d = nc.dram_tensor("w2", [U, U], F32R, kind="ExternalInput")
    b2_d = nc.dram_tensor("b2", [U], F32, kind="ExternalInput")
    v_d = nc.dram_tensor("v", [U, 1], F32R, kind="ExternalInput")
    ident_d = nc.dram_tensor("ident", [128, 128], F32R, kind="ExternalInput")
    ind_d = nc.dram_tensor("ind", [128, SUB], F32R, kind="ExternalInput")
    zeros_d = nc.dram_tensor("zeros", [128, 2 * RB // 128], F32R, kind="ExternalInput")

    ctx_d = nc.dram_tensor("ctx", [BC, D], F32, kind="ExternalOutput")
    attw_d = nc.dram_tensor("attw", [BC, L], F32, kind="ExternalOutput")

    with tile.TileContext(nc) as tc:
        with (
            tc.tile_pool(name="const", bufs=1) as const,
            tc.tile_pool(name="featp", bufs=2) as featp,
            tc.tile_pool(name="featTp", bufs=1) as featTp,
            tc.tile_pool(name="scorep", bufs=1) as scorep,
            tc.tile_pool(name="smallp", bufs=2) as smallp,
            tc.tile_pool(name="pfps", bufs=2, space="PSUM") as pfps,
            tc.tile_pool(name="tpps", bufs=1, space="PSUM") as tpps,
            tc.tile_pool(name="lgps", bufs=2, space="PSUM") as lgps,
            tc.tile_pool(name="ctxps", bufs=1, space="PSUM") as ctxps,
        ):
            # ---- constants / weights -------------------------------------
            ident = const.tile([128, 128], F32R)
            nc.sync.dma_start(out=ident, in_=ident_d.ap())
            ind = const.tile([128, SUB], F32R)
            nc.sync.dma_start(out=ind, in_=ind_d.ap())

            w1 = const.tile([128, DK, U], F32R)   # w1[p,k,u] = W1[128k+p, u]
            nc.sync.dma_start(out=w1, in_=w1_d.ap().rearrange("(k p) u -> p k u", p=128))
            w2 = const.tile([128, UK, U], F32R)   # w2[p,k,u] = W2[128k+p, u]
            nc.sync.dma_start(out=w2, in_=w2_d.ap().rearrange("(k p) u -> p k u", p=128))
            vsb = const.tile([128, UK], F32R)     # vsb[p,k] = V[128k+p, 0]
            nc.sync.dma_start(out=vsb, in_=v_d.ap().rearrange("(k p) o -> p (k o)", p=128))
            b1s = const.tile([128, UK], F32)
            nc.sync.dma_start(out=b1s, in_=b1_d.ap().rearrange("(k p) -> p k", p=128))
            b12 = const.tile([128, UK], F32)     # b12[p,k] = b1[128k+p] + b2[128k+p]
            nc.sync.dma_start(out=b12, in_=b2_d.ap().rearrange("(k p) -> p k", p=128))
            nc.vector.tensor_add(b12, b12, b1s)

            # ---- hidden projection proj_h = hidden @ W2  (no bias) -------
            hid_nat = const.tile([128, BC // 128, U], F32R)
            nc.sync.dma_start(out=hid_nat, in_=hid_d.ap().rearrange("(c p) u -> p c u", p=128))

            # hidT[p,k,b] = hidden[b, 128k+p]
            hidT = const.tile([128, UK, BC], F32R)
            for k in range(UK):
                tp = tpps.tile([128, 512], F32)
                for c in range(BC // 128):
                    nc.tensor.transpose(
                        r(tp[:, 128 * c : 128 * (c + 1)]),
                        hid_nat[:, c, 128 * k : 128 * (k + 1)],
                        ident,
                    )
                nc.vector.tensor_copy(hidT[:, k, :], tp[:, :BC])

            # projh_nat[p,c,u] = proj_h[128c+p, u]
            projh_nat = const.tile([128, BC // 128, U], F32R)
            for c in range(BC // 128):
                tp = tpps.tile([128, 512], F32)
                for k in range(UK):
                    nc.tensor.matmul(
                        tp,
                        lhsT=r(hidT[:, k, 128 * c : 128 * (c + 1)]),
                        rhs=r(w2[:, k, :]),
                        start=(k == 0),
                        stop=(k == UK - 1),
                    )
                nc.vector.tensor_copy(projh_nat[:, c, :], tp)

            # ph32[32*(g%4) + p8, g//4, u] = proj_h[8*g + p8, u]: the 8-batch
            # group g sits at 32-aligned partition base for the K=8 spread
            # matmul (PE row-strip = rhs partition base must be 32-aligned).
            ph32 = const.tile([128, BC // 8 // 4, U], F32R)
            for pd in range(16):
                for cb in range(BC // 128):
                    g = 16 * cb + pd
                    nc.sync.dma_start(
                        out=ph32[32 * (g % 4) : 32 * (g % 4) + 8, g // 4, :],
                        in_=projh_nat[8 * pd : 8 * pd + 8, cb, :],
                    )

            # context accumulator: [128, 2*BC] = (D-chunk k, batch) columns
            ctx_psum = ctxps.tile([128, 2 * BC], F32)

            # persistent block-diagonal weight buffer; the zero positions are
            # invariant across blocks, so zero once via DMA and only rewrite
            # the weight columns per block.
            blkd = const.tile([128, 2 * CPB], F32R)
            nc.sync.dma_start(out=blkd, in_=zeros_d.ap())

            # ---- main per-block pipeline ---------------------------------
            for blk in range(NBLK):
                r0 = blk * RB

                # natural features: feat_nat[p, c, d] = f[r0 + 128c + p, d]
                feat_nat = featp.tile([128, CPB, D], F32R)
                nc.sync.dma_start(
                    out=feat_nat,
                    in_=feats_d.ap()[r0 : r0 + RB, :].rearrange("(c p) d -> p c d", p=128),
                )

                # transposed features: featT[p, k, rr] = f[r0 + rr, 128k + p]
                featT = featTp.tile([128, DK, RB], F32R)
                for k in range(DK):
                    for cg in range(CPB // 4):
                        tp = tpps.tile([128, 512], F32)
                        for ci in range(4):
                            c = 4 * cg + ci
                            nc.tensor.transpose(
                                r(tp[:, 128 * ci : 128 * (ci + 1)]),
                                feat_nat[:, c, 128 * k : 128 * (k + 1)],
                                ident,
                            )
                        nc.vector.tensor_copy(featT[:, k, 512 * cg : 512 * (cg + 1)], tp)

                # scoreT[p, j, rr] = tanh(proj[128j+p, r0+rr] + b1 + b2)
                scoreT = scorep.tile([128, UK, RB], F32R)
                for j in range(UK):
                    for sp in range(NSUB // 2):
                        pf = pfps.tile([128, 2 * SUB], F32)
                        for s2 in range(2):
                            s = 2 * sp + s2
                            sl = slice(SUB * s, SUB * (s + 1))
                            out = pf[:, SUB * s2 : SUB * (s2 + 1)]
                            nc.tensor.matmul(
                                out, lhsT=r(w1[:, 0, 128 * j : 128 * (j + 1)]),
                                rhs=r(featT[:, 0, sl]), start=True, stop=False,
                            )
                            nc.tensor.matmul(
                                out, lhsT=r(w1[:, 1, 128 * j : 128 * (j + 1)]),
                                rhs=r(featT[:, 1, sl]), start=False, stop=False,
                            )
                            # spread proj_h over each batch's 64 rows (rank-8)
                            g = (r0 + SUB * s) // 512  # global 8-batch group
                            q = g % 4
                            nc.tensor.matmul(
                                out,
                                lhsT=r(ph32[32 * q : 32 * q + 8, g // 4,
                                            128 * j : 128 * (j + 1)]),
                                rhs=r(ind[32 * q : 32 * q + 8, :]),
                                start=False, stop=True,
                                tile_position=(32 * q, 0),
                            )
                        nc.scalar.activation(
                            out=scoreT[:, j, 1024 * sp : 1024 * (sp + 1)],
                            in_=pf,
                            func=mybir.ActivationFunctionType.Tanh,
                            bias=b12[:, j : j + 1],
                        )

                # logits[rr] = sum_u V[u] * scoreT[u, rr]   -> [1, RB] sbuf
                logits_sb = smallp.tile([1, RB], F32)
                for s in range(NSUB):
                    lg = lgps.tile([1, SUB], F32)
                    for j in range(UK):
                        nc.tensor.matmul(
                            lg, lhsT=r(vsb[:, j : j + 1]),
                            rhs=r(scoreT[:, j, SUB * s : SUB * (s + 1)]),
                            start=(j == 0), stop=(j == UK - 1),
                        )
                    nc.vector.tensor_copy(logits_sb[:, SUB * s : SUB * (s + 1)], lg)

                # reshape [1, 2048] -> [16, 128]: partition pp holds batches
                # (2pp, 2pp+1) of this block: cols 0:64 / 64:128
                lg_resh = smallp.tile([16, 128], F32)
                nc.sync.dma_start(out=lg_resh, in_=logits_sb)

                # softmax over each 64-col group (no max subtraction: logits
                # are bounded by ||V||_1 * max|tanh| << fp32 exp overflow)
                expw = smallp.tile([16, 128], F32)
                sums = smallp.tile([16, 2], F32)
                for h in range(2):
                    nc.scalar.activation(
                        out=expw[:, 64 * h : 64 * (h + 1)],
                        in_=lg_resh[:, 64 * h : 64 * (h + 1)],
                        func=mybir.ActivationFunctionType.Exp,
                        accum_out=sums[:, h : h + 1],
                    )
                rsums = smallp.tile([16, 2], F32)
                nc.vector.reciprocal(rsums, sums)
                attw_sb = smallp.tile([16, 128], F32R)
                for h in range(2):
                    nc.vector.tensor_scalar_mul(
                        attw_sb[:, 64 * h : 64 * (h + 1)],
                        expw[:, 64 * h : 64 * (h + 1)],
                        rsums[:, h : h + 1],
                    )
                b0 = blk * (RB // L)
                nc.sync.dma_start(
                    out=attw_d.ap()[b0 : b0 + RB // L, :]
                    .rearrange("(p h) l -> p (h l)", h=2).bitcast(F32R),
                    in_=attw_sb,
                )

                # block-diagonal weight columns: blkd[q, 2c+e] =
                #   (q//64 == e) * attw(batch 2c+e, l=q%64)
                wt = tpps.tile([128, 512], F32, tag="tp")
                nc.tensor.transpose(r(wt[:, 0:16]), attw_sb, ident[0:16, 0:16])
                blkd_v = blkd.rearrange("p (c two) -> p c two", two=2)
                nc.vector.tensor_copy(
                    blkd_v[0:64, :, 0:1], wt[0:64, 0:16].unsqueeze(2)
                )
                nc.vector.tensor_copy(
                    blkd_v[64:128, :, 1:2], wt[64:128, 0:16].unsqueeze(2)
                )

                # context: ctx_psum[:, k*BC + 2*(16blk+c) + e] =
                #   sum_r feat_nat[r, 128k+dp] * blkd[r, 2c+e]
                for c in range(CPB):
                    for k in range(DK):
                        col = k * BC + 2 * (CPB * blk + c)
                        nc.tensor.matmul(
                            ctx_psum[:, col : col + 2],
                            lhsT=r(feat_nat[:, c, 128 * k : 128 * (k + 1)]),
                            rhs=r(blkd[:, 2 * c : 2 * c + 2]),
                            start=True, stop=True,
                        )

            # ---- finalize context: drain, transpose to natural, store ----
            ctxT_sb = const.tile([128, 2 * BC], F32R)
            nc.vector.tensor_copy(ctxT_sb, ctx_psum)
            tp = tpps.tile([128, 512], F32)
            for k in range(DK):
                for h in range(BC // 128):
                    nc.tensor.transpose(
                        r(tp[:, 128 * (2 * h + k) : 128 * (2 * h + k) + 128]),
                        ctxT_sb[:, BC * k + 128 * h : BC * k + 128 * (h + 1)],
                        ident,
                    )
            ctx_nat = const.tile([128, BC // 128, D], F32R)
            nc.vector.tensor_copy(ctx_nat.rearrange("p h d -> p (h d)"), tp)
            nc.sync.dma_start(
                out=ctx_d.ap().rearrange("(h p) d -> p h d", p=128).bitcast(F32R),
                in_=ctx_nat,
            )

    nc.compile()
    return nc


_NC = None


def _get_nc():
    global _NC
    if _NC is None:
        _NC = build_kernel()
    return _NC


def _host_consts():
    ident = np.eye(128, dtype=np.float32)
    ind = np.zeros((128, SUB), dtype=np.float32)
    for q in range(4):
        for b in range(8):
            ind[32 * q + b, 64 * b : 64 * (b + 1)] = 1.0
    zeros = np.zeros((128, 2 * RB // 128), dtype=np.float32)
    return ident, ind, zeros


def kernel(features, hidden, W1, b1, W2, b2, V, bV):
    from concourse.bass_utils import run_bass_kernel_spmd

    nc = _get_nc()
    ident, ind, zeros = _host_consts()

    features = np.ascontiguousarray(features, dtype=np.float32)
    hidden = np.ascontiguousarray(hidden, dtype=np.float32)
    common = {
        "w1": np.ascontiguousarray(W1, dtype=np.float32),
        "b1": np.ascontiguousarray(b1, dtype=np.float32),
        "w2": np.ascontiguousarray(W2, dtype=np.float32),
        "b2": np.ascontiguousarray(b2, dtype=np.float32),
        "v": np.ascontiguousarray(V, dtype=np.float32),
        "ident": ident,
        "ind": ind,
        "zeros": zeros,
    }
    in_maps = []
    for i in range(N_CORES):
        sl = slice(i * BC, (i + 1) * BC)
        in_maps.append(
            {
                "feats": features[sl].reshape(ROWS, D),
                "hid": hidden[sl],
                **common,
            }
        )

    res = run_bass_kernel_spmd(nc, in_maps, list(range(N_CORES)))
    ctx = np.concatenate([res.results[i]["ctx"] for i in range(N_CORES)], axis=0)
    attw = np.concatenate([res.results[i]["attw"] for i in range(N_CORES)], axis=0)
    # softmax(x + bV) == softmax(x): bV shifts every logit in a batch equally.
    return ctx, attw[:, :, None]


# revision 9
# speedup vs baseline: 1.3207x; 1.3207x over previous
"""Bahdanau-style attention kernel for Trainium2 (Bass/Tile), 8-core data parallel.

Reference computation (per batch b):
    proj_f = features @ W1 + b1            # [B, L, U]
    proj_h = hidden @ W2 + b2              # [B, U]
    score  = tanh(proj_f + proj_h[:,None]) # [B, L, U]
    logits = score @ V + bV                # [B, L, 1]
    attw   = softmax(logits, axis=1)       # [B, L, 1]
    ctx    = sum(attw * features, axis=1)  # [B, D]

Shapes: B=2048, L=64, D=256, U=512.  Sharding: batch split across 8 cores
(Bc=256 per core, rows = Bc*L = 16384).  All matmuls run in float32r
(single-pass fp32 streaming on the PE; ~1.3e-4 relative error per contraction).

Layout strategy per core:
  - features loaded naturally [rows, D] (rows on partitions), PE-transposed to
    featT [D, rows] for the W1 matmul (PE contracts the partition dim).
  - projT [U on partitions, rows free] accumulated in PSUM: W1 chunks
    stationary, featT streams N=512 columns per matmul.
  - proj_h spread across each batch's 64 rows by a rank-8 indicator matmul
    into the same PSUM accumulation; the 8-batch groups sit at rotating
    32-aligned partition strips so consecutive spread matmuls hit different
    PE row-groups.
  - tanh on ScalarE (PSUM -> SBUF), bias carries b1+b2.
  - logits: V stationary (M=1), scoreT streams, accumulated over 4 U chunks.
  - softmax without max subtraction (logits bounded by ||V||_1, no overflow):
    exp with fused accum_out on ScalarE, reciprocal + scale on VectorE.
  - context: block-diagonal weight columns stationary ([128 rows, 2 batches],
    trivial weight load), natural features stream N=256; drains packed
    [2, 512] and split between ScalarE and VectorE.  Context matmuls for
    block k are emitted inside block k+1 (1-deep software pipeline) so the
    PE never idles waiting for the softmax chain.
"""

import sys

if "/opt/trn_rl_repo" not in sys.path:
    sys.path.insert(0, "/opt/trn_rl_repo")

import numpy as np

import concourse.bass as bass
import concourse.tile as tile
from concourse import bacc, mybir

F32 = mybir.dt.float32
F32R = mybir.dt.float32r

N_CORES = 8
B_FULL = 2048
L = 64
D = 256
U = 512

BC = B_FULL // N_CORES          # batches per core          = 256
ROWS = BC * L                   # rows per core             = 16384
RB = 2048                       # rows per block
NBLK = ROWS // RB               # blocks per core           = 8
CPB = RB // 128                 # 128-row chunks per block  = 16
SUB = 512                       # rows per matmul sub-chunk
NSUB = RB // SUB                # sub-chunks per block      = 4
DK = D // 128                   # D chunks                  = 2
UK = U // 128                   # U chunks                  = 4


def r(ap):
    return ap.bitcast(F32R)


def build_kernel():
    nc = bacc.Bacc("TRN2", target_bir_lowering=False, debug=False)

    feats_d = nc.dram_tensor("feats", [ROWS, D], F32R, kind="ExternalInput")
    hid_d = nc.dram_tensor("hid", [BC, U], F32R, kind="ExternalInput")
    w1_d = nc.dram_tensor("w1", [D, U], F32R, kind="ExternalInput")
    b1_d = nc.dram_tensor("b1", [U], F32, kind="ExternalInput")
    w2_d = nc.dram_tensor("w2", [U, U], F32R, kind="ExternalInput")
    b2_d = nc.dram_tensor("b2", [U], F32, kind="ExternalInput")
    v_d = nc.dram_tensor("v", [U, 1], F32R, kind="ExternalInput")
    ident_d = nc.dram_tensor("ident", [128, 128], F32R, kind="ExternalInput")
    ind_d = nc.dram_tensor("ind", [128, SUB], F32R, kind="ExternalInput")
    zeros_d = nc.dram_tensor("zeros", [128, 2 * CPB], F32R, kind="ExternalInput")

    ctx_d = nc.dram_tensor("ctx", [BC, D], F32, kind="ExternalOutput")
    attw_d = nc.dram_tensor("attw", [BC, L], F32, kind="ExternalOutput")

    with tile.TileContext(nc) as tc:
        with (
            tc.tile_pool(name="const", bufs=1) as const,
            tc.tile_pool(name="featp", bufs=3) as featp,
            tc.tile_pool(name="featTp", bufs=1) as featTp,
            tc.tile_pool(name="scorep", bufs=1) as scorep,
            tc.tile_pool(name="smallp", bufs=2) as smallp,
            tc.tile_pool(name="pfps", bufs=3, space="PSUM") as pfps,
            tc.tile_pool(name="tpps", bufs=2, space="PSUM") as tpps,
            tc.tile_pool(name="lgps", bufs=1, space="PSUM") as lgps,
            tc.tile_pool(name="ctxps", bufs=2, space="PSUM") as ctxps,
        ):
            # ---- constants / weights -------------------------------------
            ident = const.tile([128, 128], F32R)
            nc.sync.dma_start(out=ident, in_=ident_d.ap())
            ind = const.tile([128, SUB], F32R)
            nc.sync.dma_start(out=ind, in_=ind_d.ap())

            w1 = const.tile([128, DK, U], F32R)   # w1[p,k,u] = W1[128k+p, u]
            nc.sync.dma_start(out=w1, in_=w1_d.ap().rearrange("(k p) u -> p k u", p=128))
            w2 = const.tile([128, UK, U], F32R)   # w2[p,k,u] = W2[128k+p, u]
            nc.sync.dma_start(out=w2, in_=w2_d.ap().rearrange("(k p) u -> p k u", p=128))
            vsb = const.tile([128, UK], F32R)     # vsb[p,k] = V[128k+p, 0]
            nc.sync.dma_start(out=vsb, in_=v_d.ap().rearrange("(k p) o -> p (k o)", p=128))
            b1s = const.tile([128, UK], F32)
            nc.sync.dma_start(out=b1s, in_=b1_d.ap().rearrange("(k p) -> p k", p=128))
            b12 = const.tile([128, UK], F32)      # b12[p,k] = b1[128k+p] + b2[128k+p]
            nc.sync.dma_start(out=b12, in_=b2_d.ap().rearrange("(k p) -> p k", p=128))
            nc.vector.tensor_add(b12, b12, b1s)

            # ---- hidden projection proj_h = hidden @ W2 (no bias) --------
            hid_nat = const.tile([128, BC // 128, U], F32R)
            nc.sync.dma_start(out=hid_nat, in_=hid_d.ap().rearrange("(c p) u -> p c u", p=128))

            hidT = const.tile([128, UK, BC], F32R)  # hidT[p,k,b] = hidden[b, 128k+p]
            for k in range(UK):
                tp = tpps.tile([128, 512], F32, tag="tp")
                for c in range(BC // 128):
                    nc.tensor.transpose(
                        r(tp[:, 128 * c : 128 * (c + 1)]),
                        hid_nat[:, c, 128 * k : 128 * (k + 1)],
                        ident,
                    )
                nc.vector.tensor_copy(hidT[:, k, :], tp[:, :BC])

            # projh_nat[p,c,u] = proj_h[128c+p, u]
            projh_nat = const.tile([128, BC // 128, U], F32R)
            for c in range(BC // 128):
                tp = tpps.tile([128, 512], F32, tag="tp")
                for k in range(UK):
                    nc.tensor.matmul(
                        tp,
                        lhsT=hidT[:, k, 128 * c : 128 * (c + 1)],
                        rhs=w2[:, k, :],
                        start=(k == 0),
                        stop=(k == UK - 1),
                    )
                nc.vector.tensor_copy(projh_nat[:, c, :], tp)

            # ph32[32*(g%4) + p8, g//4, u] = proj_h[8g + p8, u]: each 8-batch
            # group g at a 32-aligned partition strip for the K=8 spread
            # matmul (strip rotates with g so adjacent spreads use different
            # PE row-groups).
            ph32 = const.tile([128, BC // 32, U], F32R)
            for pd in range(16):
                for cb in range(BC // 128):
                    g = 16 * cb + pd
                    nc.sync.dma_start(
                        out=ph32[32 * (g % 4) : 32 * (g % 4) + 8, g // 4, :],
                        in_=projh_nat[8 * pd : 8 * pd + 8, cb, :],
                    )

            # persistent double-buffered block-diagonal weights; the zero
            # columns are invariant so each buffer is zeroed once via DMA.
            blkds = []
            for i in range(2):
                blkd = const.tile([128, 2 * CPB], F32R, tag=f"blkd{i}")
                nc.sync.dma_start(out=blkd, in_=zeros_d.ap())
                blkds.append(blkd)

            # ---- context matmuls (emitted one block late: sw pipeline) ---
            def emit_ctx(state):
                feat_nat_p, blkd_p, blk_p = state
                ctx_sb = smallp.tile([2, CPB // 2, 2, D], F32, tag="ctx_sb")
                for dpair in range(CPB // 2):
                    cp = ctxps.tile([2, 512], F32, tag="cp")
                    for e in range(2):
                        c = 2 * dpair + e
                        nc.tensor.matmul(
                            cp[:, 256 * e : 256 * (e + 1)],
                            lhsT=blkd_p[:, 2 * c : 2 * c + 2],
                            rhs=feat_nat_p[:, c, :],
                            start=True, stop=True,
                        )
                    dst = ctx_sb[:, dpair, :, :]
                    if dpair % 2 == 0:
                        nc.vector.tensor_copy(dst, cp.rearrange("p (two d) -> p two d", two=2))
                    else:
                        nc.scalar.copy(dst, cp.rearrange("p (two d) -> p two d", two=2))
                # ctx_sb[e, dp, two, :] = context[batch 2*(2*dp+two)+e]
                nc.sync.dma_start(
                    out=ctx_d.ap()[blk_p * (RB // L) : (blk_p + 1) * (RB // L), :]
                    .rearrange("(dp two e) d -> e dp two d", two=2, e=2),
                    in_=ctx_sb,
                )

            prev = None

            # ---- main per-block pipeline ---------------------------------
            for blk in range(NBLK):
                r0 = blk * RB

                # natural features: feat_nat[p, c, d] = f[r0 + 128c + p, d]
                feat_nat = featp.tile([128, CPB, D], F32R)
                nc.sync.dma_start(
                    out=feat_nat,
                    in_=feats_d.ap()[r0 : r0 + RB, :].rearrange("(c p) d -> p c d", p=128),
                )

                # transposed features: featT[p, k, rr] = f[r0 + rr, 128k + p]
                featT = featTp.tile([128, DK, RB], F32R)
                for k in range(DK):
                    for cg in range(CPB // 4):
                        tp = tpps.tile([128, 512], F32, tag="tp")
                        for ci in range(4):
                            c = 4 * cg + ci
                            nc.tensor.transpose(
                                r(tp[:, 128 * ci : 128 * (ci + 1)]),
                                feat_nat[:, c, 128 * k : 128 * (k + 1)],
                                ident,
                            )
                        nc.vector.tensor_copy(featT[:, k, 512 * cg : 512 * (cg + 1)], tp)

                # scoreT[p, j, rr] = tanh(proj[128j+p, r0+rr] + b1 + b2)
                scoreT = scorep.tile([128, UK, RB], F32R)
                for j in range(UK):
                    for shalf in range(2):
                        pfs = [pfps.tile([128, SUB], F32, tag="pf", name=f"pf{i}") for i in range(2)]
                        subs = [2 * shalf, 2 * shalf + 1]
                        for k in range(DK):
                            for i, s in enumerate(subs):
                                nc.tensor.matmul(
                                    pfs[i],
                                    lhsT=w1[:, k, 128 * j : 128 * (j + 1)],
                                    rhs=featT[:, k, SUB * s : SUB * (s + 1)],
                                    start=(k == 0), stop=False,
                                )
                        # rank-8 proj_h spread; adjacent row-strips differ
                        for i, s in enumerate(subs):
                            g = (r0 + SUB * s) // 512
                            q = g % 4
                            nc.tensor.matmul(
                                pfs[i],
                                lhsT=ph32[32 * q : 32 * q + 8, g // 4,
                                          128 * j : 128 * (j + 1)],
                                rhs=ind[32 * q : 32 * q + 8, :],
                                start=False, stop=True,
                                tile_position=(32 * q, 0),
                            )
                        for i, s in enumerate(subs):
                            nc.scalar.activation(
                                out=scoreT[:, j, SUB * s : SUB * (s + 1)],
                                in_=pfs[i],
                                func=mybir.ActivationFunctionType.Tanh,
                                bias=b12[:, j : j + 1],
                            )

                # logits[rr] = sum_u V[u] * scoreT[u, rr]   -> [1, RB] sbuf
                logits_sb = smallp.tile([1, RB], F32)
                for s in range(NSUB):
                    lg = lgps.tile([1, SUB], F32)
                    for j in range(UK):
                        nc.tensor.matmul(
                            lg, lhsT=vsb[:, j : j + 1],
                            rhs=scoreT[:, j, SUB * s : SUB * (s + 1)],
                            start=(j == 0), stop=(j == UK - 1),
                        )
                    nc.vector.tensor_copy(logits_sb[:, SUB * s : SUB * (s + 1)], lg)

                # ---- context matmuls of the PREVIOUS block (pipeline) ----
                if prev is not None:
                    emit_ctx(prev)

                # reshape [1, 2048] -> [16, 128]: partition pp holds batches
                # (2pp, 2pp+1) of this block in col groups 0:64 / 64:128
                lg_resh = smallp.tile([16, 128], F32)
                nc.sync.dma_start(out=lg_resh, in_=logits_sb)

                expw = smallp.tile([16, 128], F32)
                sums = smallp.tile([16, 2], F32)
                for h in range(2):
                    nc.scalar.activation(
                        out=expw[:, 64 * h : 64 * (h + 1)],
                        in_=lg_resh[:, 64 * h : 64 * (h + 1)],
                        func=mybir.ActivationFunctionType.Exp,
                        accum_out=sums[:, h : h + 1],
                    )
                rsums = smallp.tile([16, 2], F32)
                nc.vector.reciprocal(rsums, sums)
                attw_sb = smallp.tile([16, 128], F32R)
                for h in range(2):
                    nc.vector.tensor_scalar_mul(
                        attw_sb[:, 64 * h : 64 * (h + 1)],
                        expw[:, 64 * h : 64 * (h + 1)],
                        rsums[:, h : h + 1],
                    )
                b0 = blk * (RB // L)
                nc.sync.dma_start(
                    out=attw_d.ap()[b0 : b0 + RB // L, :]
                    .rearrange("(p h) l -> p (h l)", h=2).bitcast(F32R),
                    in_=attw_sb,
                )

                # block-diagonal weight columns for the ctx matmuls:
                # blkd[q, 2c+e] = (q//64 == e) * attw(batch 2c+e, l=q%64)
                blkd = blkds[blk % 2]
                wt = tpps.tile([128, 512], F32, tag="tp")
                nc.tensor.transpose(r(wt[:, 0:16]), attw_sb, ident[0:16, 0:16])
                blkd_v = blkd.rearrange("p (c two) -> p c two", two=2)
                nc.vector.tensor_copy(
                    blkd_v[0:64, :, 0:1], wt[0:64, 0:16].unsqueeze(2)
                )
                nc.vector.tensor_copy(
                    blkd_v[64:128, :, 1:2], wt[64:128, 0:16].unsqueeze(2)
                )

                prev = (feat_nat, blkd, blk)

            emit_ctx(prev)

    nc.compile()
    return nc


_NC = None


def _get_nc():
    global _NC
    if _NC is None:
        _NC = build_kernel()
    return _NC


def _host_consts():
    ident = np.eye(128, dtype=np.float32)
    ind = np.zeros((128, SUB), dtype=np.float32)
    for q in range(4):
        for b in range(8):
            ind[32 * q + b, 64 * b : 64 * (b + 1)] = 1.0
    zeros = np.zeros((128, 2 * CPB), dtype=np.float32)
    return ident, ind, zeros


def kernel(features, hidden, W1, b1, W2, b2, V, bV):
    from concourse.bass_utils import run_bass_kernel_spmd

    nc = _get_nc()
    ident, ind, zeros = _host_consts()

    features = np.ascontiguousarray(features, dtype=np.float32)
    hidden = np.ascontiguousarray(hidden, dtype=np.float32)
    common = {
        "w1": np.ascontiguousarray(W1, dtype=np.float32),
        "b1": np.ascontiguousarray(b1, dtype=np.float32),
        "w2": np.ascontiguousarray(W2, dtype=np.float32),
        "b2": np.ascontiguousarray(b2, dtype=np.float32),
        "v": np.ascontiguousarray(V, dtype=np.float32),
        "ident": ident,
        "ind": ind,
        "zeros": zeros,
    }
    in_maps = []
    for i in range(N_CORES):
        sl = slice(i * BC, (i + 1) * BC)
        in_maps.append(
            {
                "feats": features[sl].reshape(ROWS, D),
                "hid": hidden[sl],
                **common,
            }
        )

    res = run_bass_kernel_spmd(nc, in_maps, list(range(N_CORES)))
    ctx = np.concatenate([res.results[i]["ctx"] for i in range(N_CORES)], axis=0)
    attw = np.concatenate([res.results[i]["attw"] for i in range(N_CORES)], axis=0)
    # softmax(x + bV) == softmax(x): bV shifts every logit in a batch equally.
    return ctx, attw[:, :, None]
